# revision 30
# baseline (speedup 1.0000x reference)
"""ATOCA forward — Trainium2 Bass kernel, 8 NeuronCores SPMD.

Sharding: data-parallel over the 6272 conv-samples (= flat token axis),
784 samples/core + 14-sample halo each side for the overlapping-window
attention kv. conv3 (1x1, mixes all channels) gets its input via an
on-device AllGather within each 4-core batch group.

Device layout backbone: channel-major [c, t] activations as [128, 2N]
tiles (cols [0:N] = channels 0:128; cols [N:2N] rows 0:64 = channels
128:192). Convs run as 9 shifted-tap PSUM-accumulated matmuls on
16-token strips; window attention uses block-diagonal Q stationaries
against strided 6x6 band APs of K, and per-(window,head) AV matmuls
against a gathered V^T band.
"""

import os
import time

import numpy as np
import ml_dtypes

BF = ml_dtypes.bfloat16

WS, OWS, NHEAD = 4, 6, 6
B, C, H, W = 2, 192, 224, 224
D = 32
NCORE = 8
SPB = 784
NS = SPB + 28
TS = NS * 16          # 12992 tokens incl halo
TOWN = SPB * 16       # 12544
NWR = 14

DBG = bool(int(os.environ.get("KDBG", "0")))

LAST_HW_EXEC_NS = None
_STATE = {}

# packed bf16 weight blob, AllGather-distributed (1275072 = 8 * 159384)
PBSPEC = [("wc1", (128, 3456)), ("wc2", (128, 3456)), ("wqk", (128, 768)),
          ("wv0", (128, 192)), ("wv1", (65, 192)), ("wpr", (128, 384)),
          ("wf1", (128, 768)), ("wf2", (128, 576)), ("gm", (64, 512)),
          ("csel", (128, 8))]
PBN = sum(a * b for _, (a, b) in PBSPEC)
PBSH = PBN // NCORE


def _calc_rpi():
    co = np.stack(np.meshgrid(np.arange(WS), np.arange(WS), indexing="ij")).reshape(2, -1)
    ce = np.stack(np.meshgrid(np.arange(OWS), np.arange(OWS), indexing="ij")).reshape(2, -1)
    rel = (ce[:, None, :] - co[:, :, None]).transpose(1, 2, 0) + (WS - 1)
    return rel[..., 0] * (WS + OWS - 1) + rel[..., 1]  # [16, 36]


def _pack_params(P):
    f32 = np.float32
    out = {}

    def conv_taps(w):
        m = np.zeros((128, 3456), f32)
        for t in range(9):
            di, dj = t // 3, t % 3
            lt = w[:, :, di, dj].T  # [ci, co]
            m[:, 384 * t:384 * t + 192] = lt[0:128]
            m[0:64, 384 * t + 192:384 * t + 384] = lt[128:192]
        return m

    out["wc1"] = conv_taps(P["conv1_w"]).astype(BF)
    out["wc2"] = conv_taps(P["conv2_w"]).astype(BF)

    scale = f32(D) ** f32(-0.5)
    g1, b1 = P["ln1_g"], P["ln1_b"]
    qw = P["qkv_w"]
    wq = (g1[:, None] * qw[:, 0:192]) * scale
    bq = (P["qkv_b"][0:192] + b1 @ qw[:, 0:192]) * scale
    wk = g1[:, None] * qw[:, 192:384]
    bk = P["qkv_b"][192:384] + b1 @ qw[:, 192:384]
    wv = g1[:, None] * qw[:, 384:576]
    bv = P["qkv_b"][384:576] + b1 @ qw[:, 384:576]

    m = np.zeros((128, 768), f32)
    m[:, 0:192] = wq[0:128]
    m[0:64, 192:384] = wq[128:192]
    m[:, 384:576] = wk[0:128]
    m[0:64, 576:768] = wk[128:192]
    out["wqk"] = m.astype(BF)

    out["wv0"] = wv[0:128].astype(BF)
    wv1 = np.zeros((65, 192), f32)
    wv1[0:64] = wv[128:192]
    wv1[64] = bv
    out["wv1"] = wv1.astype(BF)

    pw = P["proj_w"]
    m = np.zeros((128, 384), f32)
    m[:, 0:192] = pw[0:128]
    m[0:64, 192:384] = pw[128:192]
    out["wpr"] = m.astype(BF)

    g2, b2 = P["ln2_g"], P["ln2_b"]
    f1 = g2[:, None] * P["fc1_w"]
    bf1 = P["fc1_b"] + b2 @ P["fc1_w"]
    m = np.zeros((128, 768), f32)
    m[:, 0:384] = f1[0:128]
    m[0:64, 384:768] = f1[128:192]
    out["wf1"] = m.astype(BF)

    f2 = P["fc2_w"]
    m = np.zeros((128, 576), f32)
    for kc in range(3):
        m[:, 192 * kc:192 * kc + 192] = f2[128 * kc:128 * kc + 128]
    out["wf2"] = m.astype(BF)

    gm = np.zeros((64, 512), f32)
    cidx = np.arange(192)
    for g in range(4):
        sel = (cidx // 32 == g).astype(f32)
        gm[g, 0:128] = (sel * P["gn1_g"])[0:128]
        gm[g, 192:320] = (sel * P["gn2_g"])[0:128]
    for g in range(4, 6):
        sel = (cidx // 32 == g).astype(f32)
        gm[28 + g, 128:192] = (sel * P["gn1_g"])[128:192]
        gm[28 + g, 320:384] = (sel * P["gn2_g"])[128:192]
    gm[4, 0:128] = P["gn1_b"][0:128]
    gm[34, 128:192] = P["gn1_b"][128:192]
    gm[4, 192:320] = P["gn2_b"][0:128]
    gm[34, 320:384] = P["gn2_b"][128:192]
    gm[0, 384:512] = 1.0
    out["gm"] = gm.astype(BF)

    cs = np.zeros((128, 8), f32)
    for g in range(4):
        cs[:, g] = (np.arange(128) // 32 == g).astype(f32)
    for g in range(2):
        cs[0:64, 4 + g] = (np.arange(64) // 32 == g).astype(f32)
    cs[:, 6] = 1.0
    cs[:, 7] = 1.0
    out["csel"] = cs.astype(BF)

    rpi = _calc_rpi()
    bias = P["rpb"][rpi.reshape(-1)].reshape(16, 36, NHEAD)
    rb = np.zeros((96, 36), f32)
    for h in range(NHEAD):
        row = 16 * h if h < 4 else 64 + 16 * (h - 4)
        rb[row:row + 16, :] = bias[:, :, h]
    out["rpb96"] = rb.astype(f32)

    bias_t = np.zeros((128, 16), f32)
    bias_t[0:128, 0] = P["conv1_b"][0:128]
    bias_t[0:64, 1] = P["conv1_b"][128:192]
    bias_t[0:128, 2] = bq[0:128]
    bias_t[0:64, 3] = bq[128:192]
    bias_t[0:128, 4] = bk[0:128]
    bias_t[0:64, 5] = bk[128:192]
    bias_t[0:128, 6] = P["proj_b"][0:128]
    bias_t[0:64, 7] = P["proj_b"][128:192]
    for i in range(3):
        bias_t[:, 8 + i] = bf1[128 * i:128 * i + 128]
    bias_t[0:128, 11] = P["fc2_b"][0:128]
    bias_t[0:64, 12] = P["fc2_b"][128:192]
    bias_t[0:128, 13] = P["conv2_b"][0:128]
    bias_t[0:64, 14] = P["conv2_b"][128:192]
    out["biast"] = bias_t
    return out


def _build():
    import contextlib
    import concourse.mybir as mybir
    import concourse.tile as tile
    from concourse import bacc
    from concourse.ap import AP
    from concourse.masks import make_identity

    f32, bf16 = mybir.dt.float32, mybir.dt.bfloat16
    AF = mybir.ActivationFunctionType
    ALU = mybir.AluOpType
    AX = mybir.AxisListType

    nc = bacc.Bacc("TRN2", target_bir_lowering=False, debug=False,
                   num_devices=NCORE)

    def din(name, shape, dtype):
        return nc.dram_tensor(name, shape, dtype, kind="ExternalInput").ap()

    def dout(name, shape, dtype):
        return nc.dram_tensor(name, shape, dtype, kind="ExternalOutput").ap()

    def dscr(name, shape, dtype, dbg=False):
        if dbg and DBG:
            return nc.dram_tensor(name, shape, dtype, kind="ExternalOutput").ap()
        return nc.dram_tensor(name, shape, dtype).ap()

    xs_d = din("xs", [NS, 3072], bf16)
    pbs_d = din("pbs", [PBSH], bf16)
    wc3_d = din("wc3", [128, 96], bf16)
    rpb_d = din("rpb96", [96, 36], f32)
    bias_d = din("biast", [128, 16], f32)
    msk_d = din("msk", [128, 2], f32)

    ys_d = dout("ys", [SPB, 3072], bf16)

    if DBG:
        dmp = {n: nc.dram_tensor(n, s, bf16, kind="ExternalOutput").ap()
               for n, s in [("qd", [192, TS]), ("kd", [192, TS]),
                            ("vt", [13058, 192]), ("t3d", [192, TOWN]),
                            ("zs", [SPB, 3072]), ("cs", [2408448]),
                            ("dbg_a", [192, TS]), ("dbg_t2", [192, TOWN])]}

    def sap(t_ap, off, dims):
        return AP(t_ap.tensor, t_ap.offset + off,
                  [list(t_ap.ap[0])] + [[s, c] for s, c in dims])

    CH = [(0, 128), (128, 64)]

    with tile.TileContext(nc, trace_sim=False) as tc:
        with contextlib.ExitStack() as top:
            const = top.enter_context(tc.tile_pool(name="const", bufs=1))
            dpool = top.enter_context(
                tc.tile_pool(name="dscratch", bufs=1, space="DRAM"))
            pbin_t = dpool.tile([PBSH], bf16, tag="pbin", name="pbint")
            pbg_t = dpool.tile([PBN], bf16, tag="pbg", name="pbgt")
            nc.sync.dma_start(out=pbin_t[:], in_=pbs_d)
            nc.gpsimd.collective_compute(
                "AllGather", ALU.bypass,
                replica_groups=[[0, 1, 2, 3, 4, 5, 6, 7]],
                ins=[pbin_t[:].opt()], outs=[pbg_t[:].opt()])
            pbg = pbg_t[:]

            def cload(nm, d_ap, shape, dtype):
                t = const.tile(shape, dtype, tag=nm)
                nc.sync.dma_start(out=t[:], in_=d_ap)
                return t

            blob = {}
            _off = 0
            for _nm, (_r, _c) in PBSPEC:
                t_ = const.tile([_r, _c], bf16, tag=_nm, name=_nm + "c")
                nc.sync.dma_start(out=t_[:], in_=AP(
                    pbg.tensor, pbg.offset + _off, [[_c, _r], [1, _c]]))
                blob[_nm] = t_
                _off += _r * _c
            WC1, WC2, WQK = blob["wc1"], blob["wc2"], blob["wqk"]
            WV0, WV1, WPR = blob["wv0"], blob["wv1"], blob["wpr"]
            WF1, WF2, GM, CSEL = blob["wf1"], blob["wf2"], blob["gm"], blob["csel"]
            WC3 = cload("wc3", wc3_d, [128, 96], bf16)
            RPB = cload("rpb", rpb_d, [96, 36], f32)
            BIAS = cload("bias", bias_d, [128, 16], f32)
            MSK = cload("msk", msk_d, [128, 2], f32)
            IDT = const.tile([128, 128], bf16, tag="idt")
            make_identity(nc, IDT[:])

            big = top.enter_context(tc.tile_pool(name="big", bufs=1))
            qd_t = dpool.tile([192, TS], bf16, tag="qd", name="qdt")
            qd = qd_t[:]
            kd_t = dpool.tile([192, TS], bf16, tag="kd", name="kdt")
            kd = kd_t[:]
            vt_t = dpool.tile([13058, 192], bf16, tag="vt", name="vtt")
            vt = vt_t[:]
            t3d_t = dpool.tile([192, TOWN], bf16, tag="t3d", name="t3dt")
            t3d = t3d_t[:]
            zs_t = dpool.tile([SPB, 3072], bf16, tag="zs", name="zst")
            zs = zs_t[:]
            cs_s_t = dpool.tile([2408448], bf16, tag="cs", name="cst")
            cs_s = cs_s_t[:]
            xgin_t = dpool.tile([2408448], bf16, tag="xgin", name="xgint")
            xgin = xgin_t[:]
            xb4_t = dpool.tile([4 * 2408448], bf16, tag="xb4", name="xb4t")
            xb4 = xb4_t[:]
            A = big.tile([128, 2 * TS], bf16)       # gn1+silu tokens (shortcut)
            T2 = big.tile([128, 2 * TOWN], bf16)    # post-attention tokens

            # zero the vt pad rows (t=-1 and t=TS edge reads)
            zpad = const.tile([2, 192], bf16, tag="zpad")
            nc.vector.memset(zpad[:], 0.0)
            nc.sync.dma_start(out=vt[0:1, :], in_=zpad[0:1, :])
            nc.sync.dma_start(out=vt[12993:12994, :], in_=zpad[1:2, :])

            # ---------- conv3 side path ----------
            nc.sync.dma_start(out=xgin,
                              in_=xs_d[14:798, :].rearrange("a b -> (a b)"))
            nc.gpsimd.collective_compute(
                "AllGather", ALU.bypass,
                replica_groups=[[0, 1, 2, 3], [4, 5, 6, 7]],
                ins=[xgin.opt()], outs=[xb4.opt()])
            with contextlib.ExitStack() as s3:
                p3 = s3.enter_context(tc.tile_pool(name="c3ps", bufs=1, space="PSUM"))
                l3 = s3.enter_context(tc.tile_pool(name="c3sb", bufs=2))
                for j in range(98):
                    xt0 = l3.tile([128, 512], bf16, tag="c3x0")
                    xt1 = l3.tile([64, 512], bf16, tag="c3x1")
                    nc.sync.dma_start(out=xt0[:], in_=AP(
                        xb4.tensor, xb4.offset + 512 * j, [[50176, 128], [1, 512]]))
                    nc.sync.dma_start(out=xt1[:], in_=AP(
                        xb4.tensor, xb4.offset + 128 * 50176 + 512 * j,
                        [[50176, 64], [1, 512]]))
                    pp = p3.tile([48, 512], f32)
                    nc.tensor.matmul(pp[:], WC3[:, 0:48], xt0[:], start=True, stop=False)
                    nc.tensor.matmul(pp[:], WC3[0:64, 48:96], xt1[:], start=False, stop=True)
                    ot = l3.tile([48, 512], bf16, tag="c3o")
                    nc.scalar.activation(ot[:], pp[:], AF.Identity,
                                         bias=BIAS[0:48, 15:16])
                    nc.sync.dma_start(out=AP(cs_s.tensor, cs_s.offset + 512 * j,
                                             [[50176, 48], [1, 512]]), in_=ot[:])

            # ---------- shared conv + groupnorm helpers ----------
            def conv_block(W9, XT, xoff, nsamp, pools):
                pcv, _, _, _ = pools
                Nw = XT.shape[1] // 2
                outs = []
                taps = [4] + [t for t in range(9) if t != 4]
                for co_i, (co0, cosz) in enumerate(CH):
                    pp = pcv.tile([cosz, 512], f32, tag=f"cv{co_i}")
                    n_mm = 0
                    for t in taps:
                        di, dj = t // 3, t % 3
                        i0, i1 = max(0, 1 - di), min(4, 5 - di)
                        j0, j1 = max(0, 1 - dj), min(4, 5 - dj)
                        ni, nj = i1 - i0, j1 - j0
                        for ci, (c0, csz) in enumerate(CH):
                            lhs = W9[0:csz, 384 * t + 192 * ci + co0:
                                     384 * t + 192 * ci + co0 + cosz]
                            rhs = sap(XT[0:csz, :],
                                      (Nw if ci else 0) + xoff
                                      + 4 * (i0 + di - 1) + (j0 + dj - 1),
                                      [(16, nsamp), (4, ni), (1, nj)])
                            o = sap(pp[:], 4 * i0 + j0,
                                    [(16, nsamp), (4, ni), (1, nj)])
                            nc.tensor.matmul(o, lhs, rhs, start=(n_mm == 0),
                                             stop=(n_mm == 17),
                                             skip_group_check=True)
                            n_mm += 1
                    outs.append(pp)
                return outs

            def gn_apply(pp0, pp1, bcol0, bcol1, gmoff, nsamp, wr_dst, silu,
                         pools):
                """stats over (group, sample), apply gamma/beta (+silu).
                wr_dst(ci, csz, ap_writer) handles the output write."""
                _, pst, pbc, sbw = pools
                y0 = sbw.tile([128, 512], bf16, tag="y0")
                y1 = sbw.tile([64, 512], bf16, tag="y1")
                n16 = nsamp * 16
                nc.scalar.activation(y0[:, 0:n16], pp0[:, 0:n16], AF.Identity,
                                     bias=BIAS[:, bcol0:bcol0 + 1])
                nc.scalar.activation(y1[:, 0:n16], pp1[:, 0:n16], AF.Identity,
                                     bias=BIAS[0:64, bcol1:bcol1 + 1])
                sq0 = sbw.tile([128, 512], bf16, tag="sq0")
                sq1 = sbw.tile([64, 512], bf16, tag="sq1")
                nc.vector.tensor_mul(sq0[:, 0:n16], y0[:, 0:n16], y0[:, 0:n16])
                nc.vector.tensor_mul(sq1[:, 0:n16], y1[:, 0:n16], y1[:, 0:n16])
                st = pst.tile([128, 512], f32, tag="st")
                st2 = pst.tile([64, 512], f32, tag="st2")
                nc.tensor.matmul(st[0:4, 0:n16], CSEL[:, 0:4], y0[:, 0:n16],
                                 start=True, stop=True)
                nc.tensor.matmul(st[32:34, 0:n16], CSEL[0:64, 4:6], y1[:, 0:n16],
                                 start=True, stop=True)
                nc.tensor.matmul(st[64:68, 0:n16], CSEL[:, 0:4], sq0[:, 0:n16],
                                 start=True, stop=True)
                nc.tensor.matmul(st2[0:2, 0:n16], CSEL[0:64, 4:6], sq1[:, 0:n16],
                                 start=True, stop=True)
                sm = sbw.tile([128, 32], f32, tag="sm")
                nc.vector.tensor_reduce(sm[0:4, 0:nsamp],
                                        sap(st[0:4, :], 0, [(16, nsamp), (1, 16)]),
                                        axis=AX.X, op=ALU.add)
                nc.vector.tensor_reduce(sm[32:34, 0:nsamp],
                                        sap(st[32:34, :], 0, [(16, nsamp), (1, 16)]),
                                        axis=AX.X, op=ALU.add)
                nc.vector.tensor_reduce(sm[64:68, 0:nsamp],
                                        sap(st[64:68, :], 0, [(16, nsamp), (1, 16)]),
                                        axis=AX.X, op=ALU.add)
                nc.vector.tensor_reduce(sm[96:98, 0:nsamp],
                                        sap(st2[0:2, :], 0, [(16, nsamp), (1, 16)]),
                                        axis=AX.X, op=ALU.add)
                ns_ = nsamp
                mu = sbw.tile([64, 32], f32, tag="mu")
                var = sbw.tile([64, 32], f32, tag="var")
                mu2 = sbw.tile([64, 32], f32, tag="mu2")
                # rows 0:4 = groups 0-3, rows 32:34 = groups 4-5 (rest garbage)
                nc.vector.tensor_scalar_mul(mu[0:34, 0:ns_], sm[0:34, 0:ns_], 1.0 / 512.0)
                nc.vector.tensor_scalar_mul(var[0:32, 0:ns_], sm[64:96, 0:ns_], 1.0 / 512.0)
                nc.vector.tensor_scalar_mul(var[32:34, 0:ns_], sm[96:98, 0:ns_], 1.0 / 512.0)
                nc.vector.tensor_mul(mu2[0:34, 0:ns_], mu[0:34, 0:ns_], mu[0:34, 0:ns_])
                nc.vector.tensor_sub(var[0:34, 0:ns_], var[0:34, 0:ns_], mu2[0:34, 0:ns_])
                nc.vector.tensor_scalar_add(var[0:34, 0:ns_], var[0:34, 0:ns_], 1e-5)
                nc.scalar.activation(var[0:34, 0:ns_], var[0:34, 0:ns_], AF.Sqrt)
                r = sbw.tile([64, 32], f32, tag="r")
                nc.vector.reciprocal(r[0:34, 0:ns_], var[0:34, 0:ns_])
                rb_ = sbw.tile([64, 32], bf16, tag="rb")
                mb_ = sbw.tile([64, 32], bf16, tag="mb")
                nc.vector.tensor_copy(rb_[0:34, 0:ns_], r[0:34, 0:ns_])
                nc.vector.tensor_mul(mu2[0:34, 0:ns_], mu[0:34, 0:ns_], r[0:34, 0:ns_])
                nc.vector.tensor_scalar_mul(mu2[0:34, 0:ns_], mu2[0:34, 0:ns_], -1.0)
                nc.vector.memset(mb_[0:35, 0:ns_], 1.0)
                nc.vector.tensor_copy(mb_[0:4, 0:ns_], mu2[0:4, 0:ns_])
                nc.vector.tensor_copy(mb_[32:34, 0:ns_], mu2[32:34, 0:ns_])
                bc = pbc.tile([128, 64], f32, tag="bc")
                bc1 = pbc.tile([64, 64], f32, tag="bc1")
                # chunk0 channels <-> groups 0-3 only; chunk1 <-> groups 4-5
                nc.tensor.matmul(bc[:, 0:ns_], GM[0:4, gmoff:gmoff + 128],
                                 rb_[0:4, 0:ns_], start=True, stop=True)
                nc.tensor.matmul(bc[:, 32:32 + ns_], GM[0:5, gmoff:gmoff + 128],
                                 mb_[0:5, 0:ns_], start=True, stop=True)
                nc.tensor.matmul(bc1[:, 0:ns_], GM[32:34, gmoff + 128:gmoff + 192],
                                 rb_[32:34, 0:ns_], start=True, stop=True)
                nc.tensor.matmul(bc1[:, 32:32 + ns_], GM[32:35, gmoff + 128:gmoff + 192],
                                 mb_[32:35, 0:ns_], start=True, stop=True)
                for ci, (bct, yt, csz) in enumerate([(bc, y0, 128), (bc1, y1, 64)]):
                    tmp = sbw.tile([csz, 512], f32, tag=f"tmp{ci}")
                    nc.vector.tensor_mul(
                        sap(tmp[:], 0, [(16, ns_), (1, 16)]),
                        sap(yt[:], 0, [(16, ns_), (1, 16)]),
                        sap(bct[:], 0, [(1, ns_), (0, 16)]))
                    nc.vector.tensor_add(
                        sap(tmp[:], 0, [(16, ns_), (1, 16)]),
                        sap(tmp[:], 0, [(16, ns_), (1, 16)]),
                        sap(bct[:], 32, [(1, ns_), (0, 16)]))
                    wr_dst(ci, csz, tmp[:, 0:n16], silu)

            # ---------- stage A: conv1 + gn1 + silu -> A ----------
            NB = (TS + 511) // 512
            with contextlib.ExitStack() as sA:
                xs_p = sA.enter_context(tc.tile_pool(name="xsp", bufs=1))
                XS = xs_p.tile([128, 2 * TS], bf16)
                for ci, (c0, csz) in enumerate(CH):
                    nc.sync.dma_start(
                        out=sap(XS[0:csz, :], (TS if ci else 0), [(16, NS), (1, 16)]),
                        in_=AP(xs_d.tensor, xs_d.offset + 16 * c0,
                               [[16, csz], [3072, NS], [1, 16]]))
                pcv = sA.enter_context(tc.tile_pool(name="Aps", bufs=2, space="PSUM"))
                pst = sA.enter_context(tc.tile_pool(name="Ast", bufs=1, space="PSUM"))
                pbc = sA.enter_context(tc.tile_pool(name="Abc", bufs=1, space="PSUM"))
                sbw = sA.enter_context(tc.tile_pool(name="Asb", bufs=3))
                pools = (pcv, pst, pbc, sbw)
                for b in range(NB):
                    t0 = 512 * b
                    nsamp = min(32, NS - 32 * b)

                    def wr_a(ci, csz, src, silu, t0=t0):
                        do = (TS if ci else 0) + t0
                        n16 = src.shape[1]
                        nc.scalar.activation(A[0:csz, do:do + n16], src, AF.Silu)

                    pp0, pp1 = conv_block(WC1, XS, t0, nsamp, pools)
                    gn_apply(pp0, pp1, 0, 1, 0, nsamp, wr_a, True, pools)

            if DBG:
                nc.sync.dma_start(out=dmp["dbg_a"][0:128, :], in_=A[:, 0:TS])
                nc.sync.dma_start(out=dmp["dbg_a"][128:192, :], in_=A[0:64, TS:2 * TS])

            # ---------- stage 5: ln1 + q/k GEMMs + V^T ----------
            with contextlib.ExitStack() as s5:
                xn_p = s5.enter_context(tc.tile_pool(name="xnp", bufs=1))
                XN = xn_p.tile([128, 2 * TS], bf16)
                nc.vector.memset(XN[64:65, TS:2 * TS], 1.0)
                ps5 = s5.enter_context(tc.tile_pool(name="p5", bufs=1, space="PSUM"))
                ps5b = s5.enter_context(tc.tile_pool(name="p5b", bufs=1, space="PSUM"))
                sb5 = s5.enter_context(tc.tile_pool(name="sb5", bufs=2))

                for b in range(NB):
                    t0 = 512 * b
                    ncols = min(512, TS - t0)
                    sq0 = sb5.tile([128, 512], bf16, tag="sq0")
                    sq1 = sb5.tile([64, 512], bf16, tag="sq1")
                    nc.vector.tensor_mul(sq0[:, 0:ncols], A[:, t0:t0 + ncols],
                                         A[:, t0:t0 + ncols])
                    nc.vector.tensor_mul(sq1[:, 0:ncols],
                                         A[0:64, TS + t0:TS + t0 + ncols],
                                         A[0:64, TS + t0:TS + t0 + ncols])
                    st = ps5.tile([64, 512], f32, tag="lnst")
                    nc.tensor.matmul(st[0:1, 0:ncols], CSEL[:, 6:7],
                                     A[:, t0:t0 + ncols], start=True, stop=False)
                    nc.tensor.matmul(st[0:1, 0:ncols], CSEL[0:64, 7:8],
                                     A[0:64, TS + t0:TS + t0 + ncols],
                                     start=False, stop=True)
                    nc.tensor.matmul(st[32:33, 0:ncols], CSEL[:, 6:7], sq0[:, 0:ncols],
                                     start=True, stop=False)
                    nc.tensor.matmul(st[32:33, 0:ncols], CSEL[0:64, 7:8], sq1[:, 0:ncols],
                                     start=False, stop=True)
                    mu = sb5.tile([1, 512], f32, tag="mu")
                    var = sb5.tile([1, 512], f32, tag="var")
                    mu2 = sb5.tile([1, 512], f32, tag="mu2")
                    nc.vector.tensor_scalar_mul(mu[:, 0:ncols], st[0:1, 0:ncols], 1.0 / 192.0)
                    nc.vector.tensor_scalar_mul(var[:, 0:ncols], st[32:33, 0:ncols], 1.0 / 192.0)
                    nc.vector.tensor_mul(mu2[:, 0:ncols], mu[:, 0:ncols], mu[:, 0:ncols])
                    nc.vector.tensor_sub(var[:, 0:ncols], var[:, 0:ncols], mu2[:, 0:ncols])
                    nc.vector.tensor_scalar_add(var[:, 0:ncols], var[:, 0:ncols], 1e-5)
                    nc.scalar.activation(var[:, 0:ncols], var[:, 0:ncols], AF.Sqrt)
                    r_ = sb5.tile([1, 512], f32, tag="r")
                    nc.vector.reciprocal(r_[:, 0:ncols], var[:, 0:ncols])
                    rmbr = sb5.tile([1, 512], bf16, tag="rmbr")
                    rmbm = sb5.tile([1, 512], bf16, tag="rmbm")
                    nc.vector.tensor_copy(rmbr[:, 0:ncols], r_[:, 0:ncols])
                    nc.vector.tensor_mul(mu2[:, 0:ncols], mu[:, 0:ncols], r_[:, 0:ncols])
                    nc.vector.tensor_scalar_mul(mu2[:, 0:ncols], mu2[:, 0:ncols], -1.0)
                    nc.vector.tensor_copy(rmbm[:, 0:ncols], mu2[:, 0:ncols])
                    bc = ps5b.tile([128, 1024], f32, tag="lnbc")
                    nc.tensor.matmul(bc[:, 0:ncols], GM[0:1, 384:512],
                                     rmbr[:, 0:ncols], start=True, stop=True)
                    nc.tensor.matmul(bc[:, 512:512 + ncols], GM[0:1, 384:512],
                                     rmbm[:, 0:ncols], start=True, stop=True)
                    for ci, (c0, csz) in enumerate(CH):
                        ao = (TS if ci else 0) + t0
                        nc.vector.tensor_mul(XN[0:csz, ao:ao + ncols],
                                             A[0:csz, ao:ao + ncols],
                                             bc[0:csz, 0:ncols])
                        nc.vector.tensor_add(XN[0:csz, ao:ao + ncols],
                                             XN[0:csz, ao:ao + ncols],
                                             bc[0:csz, 512:512 + ncols])
                    nc.vector.memset(XN[64:65, TS + t0:TS + t0 + ncols], 1.0)
                    # q/k gemms
                    for gi, (wcol, bcol, dst) in enumerate([(0, 2, qd), (384, 4, kd)]):
                        for co_i, (co0, cosz) in enumerate(CH):
                            pq = ps5b.tile([cosz, 512], f32, tag=f"pq{gi}{co_i}")
                            nc.tensor.matmul(
                                pq[:, 0:ncols], WQK[:, wcol + co0:wcol + co0 + cosz],
                                XN[:, t0:t0 + ncols], start=True, stop=False)
                            nc.tensor.matmul(
                                pq[:, 0:ncols],
                                WQK[0:64, wcol + 192 + co0:wcol + 192 + co0 + cosz],
                                XN[0:64, TS + t0:TS + t0 + ncols],
                                start=False, stop=True)
                            qt = sb5.tile([cosz, 512], bf16, tag=f"qt{gi}{co_i}")
                            nc.scalar.activation(
                                qt[:, 0:ncols], pq[:, 0:ncols], AF.Identity,
                                bias=BIAS[0:cosz, bcol + co_i:bcol + co_i + 1])
                            if gi == 1:
                                if t0 < 224:
                                    e = min(ncols, 224 - t0)
                                    nc.vector.tensor_scalar_mul(
                                        qt[:, 0:e], qt[:, 0:e], MSK[0:cosz, 0:1])
                                if t0 + ncols > TS - 224:
                                    s = max(0, TS - 224 - t0)
                                    nc.vector.tensor_scalar_mul(
                                        qt[:, s:ncols], qt[:, s:ncols],
                                        MSK[0:cosz, 1:2])
                            nc.sync.dma_start(out=dst[co0:co0 + cosz, t0:t0 + ncols],
                                              in_=qt[:, 0:ncols])
                    # V^T gemm
                    nt = (ncols + 127) // 128
                    for s in range(nt):
                        mt = min(128, ncols - 128 * s)
                        tok0 = t0 + 128 * s
                        pv = ps5.tile([128, 192], f32, tag="pv")
                        nc.tensor.matmul(pv[0:mt, :], XN[:, tok0:tok0 + mt],
                                         WV0[:], start=True, stop=False)
                        nc.tensor.matmul(pv[0:mt, :],
                                         XN[0:65, TS + tok0:TS + tok0 + mt],
                                         WV1[:], start=False, stop=True)
                        vtile = sb5.tile([128, 192], bf16, tag="vtile")
                        nc.scalar.activation(vtile[0:mt, :], pv[0:mt, :], AF.Copy)
                        ti = tok0 // 128
                        vm = {0: (0, 128, 0), 1: (0, 96, 0), 99: (96, 128, 1),
                              100: (0, 128, 1), 101: (0, 64, 1)}.get(ti)
                        if vm is not None:
                            r0, r1, mc = vm
                            r1 = min(r1, mt)
                            if r1 > r0:
                                nc.vector.tensor_scalar_mul(
                                    vtile[r0:r1, :], vtile[r0:r1, :],
                                    MSK[r0:r1, mc:mc + 1])
                        nc.sync.dma_start(out=vt[1 + tok0:1 + tok0 + mt, :],
                                          in_=vtile[0:mt, :])

            # ---------- stage 6: windowed attention + proj -> T2 ----------
            BW = 1348
            with contextlib.ExitStack() as s6:
                sb6 = s6.enter_context(tc.tile_pool(name="sb6", bufs=1))
                psc = s6.enter_context(tc.tile_pool(name="psc", bufs=1, space="PSUM"))
                pav = s6.enter_context(tc.tile_pool(name="pav", bufs=1, space="PSUM"))
                ppj = s6.enter_context(tc.tile_pool(name="ppj", bufs=1, space="PSUM"))

                for wr in range(NWR):
                    t0 = 896 * wr
                    KB = sb6.tile([128, 2 * BW], bf16, tag="kb")
                    for ci, (c0, csz) in enumerate(CH):
                        nc.sync.dma_start(
                            out=KB[0:csz, BW * ci + 1:BW * ci + 1345],
                            in_=kd[c0:c0 + csz, t0:t0 + 1344])
                    QB = sb6.tile([128, 2 * 896], bf16, tag="qb")
                    for ci, (c0, csz) in enumerate(CH):
                        nc.sync.dma_start(
                            out=QB[0:csz, 896 * ci:896 * ci + 896],
                            in_=qd[c0:c0 + csz, t0 + 224:t0 + 1120])
                    VB = sb6.tile([36, 56 * 192], bf16, tag="vb")
                    for ki in range(6):
                        nc.sync.dma_start(
                            out=VB[6 * ki:6 * ki + 6, :],
                            in_=AP(vt.tensor,
                                   vt.offset + 192 * (t0 + 224 * ki),
                                   [[192, 6], [768, 56], [1, 192]]))

                    QS = sb6.tile([128, 64 * 56], bf16, tag="qs")
                    QS2 = sb6.tile([64, 32 * 56], bf16, tag="qs2")
                    nc.vector.memset(QS[:], 0.0)
                    nc.vector.memset(QS2[:], 0.0)
                    for h in range(6):
                        if h < 4:
                            src = sap(QB[32 * h:32 * h + 32, :], 0,
                                      [(4, 56), (224, 4), (1, 4)])
                            dst = sap(QS[32 * h:32 * h + 32, :], 16 * h,
                                      [(64, 56), (4, 4), (1, 4)])
                        else:
                            m = h - 4
                            src = sap(QB[32 * m:32 * m + 32, :], 896,
                                      [(4, 56), (224, 4), (1, 4)])
                            dst = sap(QS2[32 * m:32 * m + 32, :], 16 * m,
                                      [(32, 56), (4, 4), (1, 4)])
                        nc.vector.tensor_copy(dst, src)

                    SC = psc.tile([96, 56 * 36], f32, tag="sc")
                    for w in range(56):
                        rhs0 = sap(KB[:], 4 * w, [(224, 6), (1, 6)])
                        rhs1 = sap(KB[0:64, :], BW + 4 * w, [(224, 6), (1, 6)])
                        nc.tensor.matmul(SC[0:64, 36 * w:36 * w + 36],
                                         QS[:, 64 * w:64 * w + 64], rhs0,
                                         start=True, stop=True)
                        nc.tensor.matmul(SC[64:96, 36 * w:36 * w + 36],
                                         QS2[:, 32 * w:32 * w + 32], rhs1,
                                         start=True, stop=True)
                    scp = SC[:].ap[0][0]
                    nc.vector.memset(
                        AP(SC[:].tensor, SC[:].offset, [[scp, 96], [6, 6]]), 0.0)
                    nc.vector.memset(
                        AP(SC[:].tensor, SC[:].offset + 36 * 55 + 5,
                           [[scp, 96], [6, 6]]), 0.0)
                    nc.vector.tensor_add(
                        sap(SC[:], 0, [(36, 56), (1, 36)]),
                        sap(SC[:], 0, [(36, 56), (1, 36)]),
                        AP(RPB[:].tensor, RPB[:].offset,
                           [list(RPB[:].ap[0]), [0, 56], [1, 36]]))
                    AS = sb6.tile([96, 56 * 36], bf16, tag="as")
                    nc.scalar.activation(AS[:], SC[:], AF.Exp)
                    SM = sb6.tile([96, 56], f32, tag="smx")
                    nc.vector.tensor_reduce(
                        SM[:], sap(AS[:], 0, [(36, 56), (1, 36)]),
                        axis=AX.X, op=ALU.add)
                    asp = AS[:].ap[0][0]
                    nc.vector.memset(
                        AP(AS[:].tensor, AS[:].offset, [[asp, 96], [6, 6]]), 0.0)
                    nc.vector.memset(
                        AP(AS[:].tensor, AS[:].offset + 36 * 55 + 5,
                           [[asp, 96], [6, 6]]), 0.0)
                    nc.vector.reciprocal(SM[:], SM[:])
                    RS = sb6.tile([96, 56], bf16, tag="rsx")
                    nc.vector.tensor_copy(RS[:], SM[:])
                    nc.vector.tensor_mul(
                        sap(AS[:], 0, [(36, 56), (1, 36)]),
                        sap(AS[:], 0, [(36, 56), (1, 36)]),
                        sap(RS[:], 0, [(1, 56), (0, 36)]))

                    ATS = sb6.tile([36, 96 * 56], bf16, tag="ats")
                    for w in range(56):
                        atp = pav.tile([128, 96], bf16, tag="avpA")
                        nc.tensor.transpose(atp[0:36, :],
                                            AS[:, 36 * w:36 * w + 36],
                                            IDT[0:96, 0:96])
                        nc.scalar.copy(ATS[0:36, 96 * w:96 * w + 96],
                                       atp[0:36, :])

                    for w8 in range(7):
                        avpA = pav.tile([128, 128], f32, tag="avpA")
                        avpB = pav.tile([128, 128], f32, tag="avpB")
                        for wl in range(8):
                            w = 8 * w8 + wl
                            for h in range(6):
                                hcol = 16 * h if h < 4 else 64 + 16 * (h - 4)
                                rhs = ATS[0:36,
                                          96 * w + hcol:96 * w + hcol + 16]
                                lhs = VB[:, 192 * w + 32 * h:192 * w + 32 * h + 32]
                                pt_ = avpA if h < 3 else avpB
                                r0 = 32 * (h % 3)
                                o = pt_[r0:r0 + 32, 16 * wl:16 * wl + 16]
                                nc.tensor.matmul(o, lhs, rhs, start=True, stop=True)
                        AO = sb6.tile([128, 256], bf16, tag="ao")
                        nc.scalar.copy(AO[0:96, 0:128], avpA[0:96, :])
                        nc.scalar.copy(AO[96:128, 0:128], avpB[0:32, :])
                        nc.scalar.copy(AO[0:32, 128:256], avpB[32:64, :])
                        nc.scalar.copy(AO[32:64, 128:256], avpB[64:96, :])
                        for co_i, (co0, cosz) in enumerate(CH):
                            pp = ppj.tile([cosz, 128], f32, tag=f"pj{co_i}")
                            nc.tensor.matmul(pp[:], WPR[:, co0:co0 + cosz],
                                             AO[:, 0:128], start=True, stop=False)
                            nc.tensor.matmul(pp[:],
                                             WPR[0:64, 192 + co0:192 + co0 + cosz],
                                             AO[0:64, 128:256], start=False, stop=True)
                            t2o = (TOWN if co_i else 0) + 896 * wr + 32 * w8
                            ao_ = (TS if co_i else 0) + t0 + 224 + 32 * w8
                            for qi in range(4):
                                nc.vector.scalar_tensor_tensor(
                                    sap(T2[0:cosz, :], t2o + 224 * qi,
                                        [(4, 8), (1, 4)]),
                                    sap(pp[:], 4 * qi, [(16, 8), (1, 4)]),
                                    BIAS[0:cosz, 6 + co_i:7 + co_i],
                                    sap(A[0:cosz, :], ao_ + 224 * qi,
                                        [(4, 8), (1, 4)]),
                                    op0=ALU.add, op1=ALU.add)

            if DBG:
                nc.sync.dma_start(out=dmp["dbg_t2"][0:128, :], in_=T2[:, 0:TOWN])
                nc.sync.dma_start(out=dmp["dbg_t2"][128:192, :],
                                  in_=T2[0:64, TOWN:2 * TOWN])

            # ---------- stage 7: ln2 + mlp -> t3d ----------
            NB2 = (TOWN + 511) // 512
            with contextlib.ExitStack() as s7:
                ps7 = s7.enter_context(tc.tile_pool(name="p7", bufs=1, space="PSUM"))
                sb7 = s7.enter_context(tc.tile_pool(name="sb7", bufs=3))
                for b in range(NB2):
                    t0 = 512 * b
                    ncols = min(512, TOWN - t0)
                    XMt = sb7.tile([128, 1024], bf16, tag="xmt")
                    sq0 = sb7.tile([128, 512], bf16, tag="sq0")
                    sq1 = sb7.tile([64, 512], bf16, tag="sq1")
                    nc.vector.tensor_mul(sq0[:, 0:ncols], T2[:, t0:t0 + ncols],
                                         T2[:, t0:t0 + ncols])
                    nc.vector.tensor_mul(sq1[:, 0:ncols],
                                         T2[0:64, TOWN + t0:TOWN + t0 + ncols],
                                         T2[0:64, TOWN + t0:TOWN + t0 + ncols])
                    st = ps7.tile([64, 512], f32, tag="lnst")
                    nc.tensor.matmul(st[0:1, 0:ncols], CSEL[:, 6:7],
                                     T2[:, t0:t0 + ncols], start=True, stop=False)
                    nc.tensor.matmul(st[0:1, 0:ncols], CSEL[0:64, 7:8],
                                     T2[0:64, TOWN + t0:TOWN + t0 + ncols],
                                     start=False, stop=True)
                    nc.tensor.matmul(st[32:33, 0:ncols], CSEL[:, 6:7], sq0[:, 0:ncols],
                                     start=True, stop=False)
                    nc.tensor.matmul(st[32:33, 0:ncols], CSEL[0:64, 7:8],
                                     sq1[:, 0:ncols], start=False, stop=True)
                    mu = sb7.tile([1, 512], f32, tag="mu")
                    var = sb7.tile([1, 512], f32, tag="var")
                    mu2 = sb7.tile([1, 512], f32, tag="mu2")
                    nc.vector.tensor_scalar_mul(mu[:, 0:ncols], st[0:1, 0:ncols], 1.0 / 192.0)
                    nc.vector.tensor_scalar_mul(var[:, 0:ncols], st[32:33, 0:ncols], 1.0 / 192.0)
                    nc.vector.tensor_mul(mu2[:, 0:ncols], mu[:, 0:ncols], mu[:, 0:ncols])
                    nc.vector.tensor_sub(var[:, 0:ncols], var[:, 0:ncols], mu2[:, 0:ncols])
                    nc.vector.tensor_scalar_add(var[:, 0:ncols], var[:, 0:ncols], 1e-5)
                    nc.scalar.activation(var[:, 0:ncols], var[:, 0:ncols], AF.Sqrt)
                    r_ = sb7.tile([1, 512], f32, tag="r")
                    nc.vector.reciprocal(r_[:, 0:ncols], var[:, 0:ncols])
                    rmbr = sb7.tile([1, 512], bf16, tag="rmbr")
                    rmbm = sb7.tile([1, 512], bf16, tag="rmbm")
                    nc.vector.tensor_copy(rmbr[:, 0:ncols], r_[:, 0:ncols])
                    nc.vector.tensor_mul(mu2[:, 0:ncols], mu[:, 0:ncols], r_[:, 0:ncols])
                    nc.vector.tensor_scalar_mul(mu2[:, 0:ncols], mu2[:, 0:ncols], -1.0)
                    nc.vector.tensor_copy(rmbm[:, 0:ncols], mu2[:, 0:ncols])
                    bc = ps7.tile([128, 1024], f32, tag="lnbc")
                    nc.tensor.matmul(bc[:, 0:ncols], GM[0:1, 384:512],
                                     rmbr[:, 0:ncols], start=True, stop=True)
                    nc.tensor.matmul(bc[:, 512:512 + ncols], GM[0:1, 384:512],
                                     rmbm[:, 0:ncols], start=True, stop=True)
                    for ci, (c0, csz) in enumerate(CH):
                        to = (TOWN if ci else 0) + t0
                        xo = 512 * ci
                        nc.vector.tensor_mul(XMt[0:csz, xo:xo + ncols],
                                             T2[0:csz, to:to + ncols],
                                             bc[0:csz, 0:ncols])
                        nc.vector.tensor_add(XMt[0:csz, xo:xo + ncols],
                                             XMt[0:csz, xo:xo + ncols],
                                             bc[0:csz, 512:512 + ncols])
                    nc.vector.memset(XMt[64:65, 512:512 + ncols], 1.0)
                    H1 = sb7.tile([128, 3 * 512], bf16, tag="h1")
                    for ft in range(3):
                        pf = ps7.tile([128, 512], f32, tag=f"pf{ft}")
                        nc.tensor.matmul(pf[:, 0:ncols],
                                         WF1[:, 128 * ft:128 * ft + 128],
                                         XMt[:, 0:ncols], start=True, stop=False)
                        nc.tensor.matmul(pf[:, 0:ncols],
                                         WF1[0:64, 384 + 128 * ft:384 + 128 * ft + 128],
                                         XMt[0:64, 512:512 + ncols],
                                         start=False, stop=True)
                        nc.scalar.activation(H1[:, 512 * ft:512 * ft + ncols],
                                             pf[:, 0:ncols], AF.Gelu,
                                             bias=BIAS[:, 8 + ft:9 + ft])
                    for co_i, (co0, cosz) in enumerate(CH):
                        pt = ps7.tile([cosz, 512], f32, tag=f"pt{co_i}")
                        for kc in range(3):
                            nc.tensor.matmul(
                                pt[:, 0:ncols],
                                WF2[:, 192 * kc + co0:192 * kc + co0 + cosz],
                                H1[:, 512 * kc:512 * kc + ncols],
                                start=(kc == 0), stop=(kc == 2))
                        to = (TOWN if co_i else 0) + t0
                        t3t = sb7.tile([cosz, 512], bf16, tag=f"t3t{co_i}")
                        nc.vector.scalar_tensor_tensor(
                            t3t[:, 0:ncols], pt[:, 0:ncols],
                            BIAS[0:cosz, 11 + co_i:12 + co_i],
                            T2[0:cosz, to:to + ncols],
                            op0=ALU.add, op1=ALU.add)
                        nc.sync.dma_start(out=t3d[co0:co0 + cosz, t0:t0 + ncols],
                                          in_=t3t[:, 0:ncols])

            # ---------- stage 8: conv2 + gn2 -> zs ----------
            with contextlib.ExitStack() as s8:
                pcv = s8.enter_context(tc.tile_pool(name="Bps", bufs=2, space="PSUM"))
                pst = s8.enter_context(tc.tile_pool(name="Bst", bufs=1, space="PSUM"))
                pbc = s8.enter_context(tc.tile_pool(name="Bbc", bufs=1, space="PSUM"))
                sbw = s8.enter_context(tc.tile_pool(name="Bsb", bufs=3))
                pools = (pcv, pst, pbc, sbw)
                for b in range(NB2):
                    t0 = 512 * b
                    nsamp = min(32, SPB - 32 * b)
                    ncols = nsamp * 16
                    T3B = sbw.tile([128, 1024], bf16, tag="t3b")
                    nc.sync.dma_start(out=T3B[:, 0:ncols],
                                      in_=t3d[0:128, t0:t0 + ncols])
                    nc.sync.dma_start(out=T3B[0:64, 512:512 + ncols],
                                      in_=t3d[128:192, t0:t0 + ncols])
                    Zt = sbw.tile([128, 1024], bf16, tag="zz")

                    def wr_z(ci, csz, src, silu, Zt=Zt):
                        nc.vector.tensor_copy(
                            Zt[0:csz, 512 * ci:512 * ci + src.shape[1]], src)

                    pp0, pp1 = conv_block(WC2, T3B, 0, nsamp, pools)
                    gn_apply(pp0, pp1, 13, 14, 192, nsamp, wr_z, False, pools)
                    for ci, (c0, csz) in enumerate(CH):
                        nc.sync.dma_start(
                            out=AP(zs.tensor, zs.offset + 3072 * 32 * b + 16 * c0,
                                   [[16, csz], [3072, nsamp], [1, 16]]),
                            in_=sap(Zt[0:csz, :], 512 * ci, [(16, nsamp), (1, 16)]))

            # ---------- stage 9: final add + relu ----------
            with contextlib.ExitStack() as s9:
                sb9 = s9.enter_context(tc.tile_pool(name="sb9", bufs=3))
                for ti in range(7):
                    r0 = 112 * ti
                    zt = sb9.tile([112, 3072], bf16, tag="zt")
                    ct = sb9.tile([112, 3072], bf16, tag="ct")
                    nc.sync.dma_start(out=zt[:], in_=zs[r0:r0 + 112, :])
                    nc.sync.dma_start(out=ct[:], in_=AP(
                        cs_s.tensor, cs_s.offset + 3072 * r0,
                        [[3072, 112], [1, 3072]]))
                    ot = sb9.tile([112, 3072], bf16, tag="ot")
                    nc.vector.tensor_add(ot[:], zt[:], ct[:])
                    nc.scalar.activation(ot[:], ot[:], AF.Relu)
                    nc.sync.dma_start(out=ys_d[r0:r0 + 112, :], in_=ot[:])

            if DBG:
                nc.sync.dma_start(out=dmp["qd"], in_=qd)
                nc.sync.dma_start(out=dmp["kd"], in_=kd)
                nc.sync.dma_start(out=dmp["vt"], in_=vt)
                nc.sync.dma_start(out=dmp["t3d"], in_=t3d)
                nc.sync.dma_start(out=dmp["zs"], in_=zs)
                nc.sync.dma_start(out=dmp["cs"], in_=cs_s)

    nc.compile()
    return nc


def _prep_inputs(inputs):
    P = {k: np.asarray(v, np.float32) for k, v in inputs.items()}
    packed = _pack_params(P)
    x = P["x"].reshape(6272, 3072)
    w3 = P["conv3_w"][:, :, 0, 0]

    blob = np.concatenate([np.ascontiguousarray(packed[nm]).reshape(-1)
                           for nm, _ in PBSPEC])
    assert blob.size == PBN
    shared = {k: v for k, v in packed.items()
              if k in ("rpb96", "biast")}

    def core_map(i):
        b = i // 4
        lo, hi = 784 * i - 14, 784 * i + 798
        xs = np.zeros((NS, 3072), np.float32)
        blo, bhi = 3136 * b, 3136 * (b + 1)
        s0, s1 = max(lo, blo), min(hi, bhi)
        xs[s0 - lo:s1 - lo] = x[s0:s1]
        m = dict(shared)
        m["pbs"] = blob[PBSH * i:PBSH * (i + 1)]
        bias_t = packed["biast"].copy()
        bias_t[0:48, 15] = P["conv3_b"][48 * (i % 4):48 * (i % 4) + 48]
        m["biast"] = bias_t
        wc3 = np.zeros((128, 96), np.float32)
        w3s = w3[48 * (i % 4):48 * (i % 4) + 48, :].T
        wc3[:, 0:48] = w3s[0:128]
        wc3[0:64, 48:96] = w3s[128:192]
        m["wc3"] = wc3.astype(BF)
        msk = np.zeros((128, 2), np.float32)
        msk[:, 0] = 0.0 if lo < blo else 1.0
        msk[:, 1] = 0.0 if hi > bhi else 1.0
        m["msk"] = msk
        m["xs"] = xs.astype(BF)
        return m

    from concurrent.futures import ThreadPoolExecutor
    with ThreadPoolExecutor(NCORE) as ex:
        in_maps = list(ex.map(core_map, range(NCORE)))
    return in_maps


def _enable_jax_cache():
    try:
        import jax
        jax.config.update("jax_compilation_cache_dir", "/tmp/atoca_jax_cache")
        jax.config.update("jax_persistent_cache_min_entry_size_bytes", -1)
        jax.config.update("jax_persistent_cache_min_compile_time_secs", 0.0)
    except Exception:
        pass


def _run_nodonate(nc, in_maps):
    """run_bass_via_pjrt minus the donated zero output buffers (the kernel
    writes every output element, so the 38MB zeros upload is pure waste)."""
    import jax
    import concourse.mybir as mybir
    from concourse import bass2jax
    from concourse.bass2jax import _bass_exec_p, partition_id_tensor
    from jax.experimental.shard_map import shard_map
    from jax.sharding import Mesh, PartitionSpec

    bass2jax.install_neuronx_cc_hook()
    partition_name = (nc.partition_id_tensor.name
                      if nc.partition_id_tensor else None)
    in_names, out_names, out_avals = [], [], []
    for alloc in nc.m.functions[0].allocations:
        if not isinstance(alloc, mybir.MemoryLocationSet):
            continue
        name = alloc.memorylocations[0].name
        if alloc.kind == "ExternalInput":
            if name != partition_name:
                in_names.append(name)
        elif alloc.kind == "ExternalOutput":
            out_names.append(name)
            out_avals.append(jax.core.ShapedArray(
                tuple(alloc.tensor_shape), mybir.dt.np(alloc.dtype)))
    all_in = list(in_names)
    if partition_name is not None:
        all_in.append(partition_name)

    def _body(*args):
        operands = list(args)
        if partition_name is not None:
            operands.append(partition_id_tensor())
        return tuple(_bass_exec_p.bind(
            *operands, out_avals=tuple(out_avals), in_names=tuple(all_in),
            out_names=tuple(out_names), lowering_input_output_aliases=(),
            sim_require_finite=True, sim_require_nnan=True, nc=nc))

    devices = jax.devices()[:NCORE]
    mesh = Mesh(np.array(devices), ("core",))
    fn = jax.jit(shard_map(
        _body, mesh=mesh, in_specs=(PartitionSpec("core"),) * len(in_names),
        out_specs=(PartitionSpec("core"),) * len(out_names), check_rep=False))
    staged = _STATE.get("staged_inputs") or {}
    concat_in = [staged[nm] if nm in staged else
                 np.concatenate([np.asarray(in_maps[c][nm])
                                 for c in range(NCORE)], axis=0)
                 for nm in in_names]
    out_arrs = fn(*concat_in)
    results = []
    for c in range(NCORE):
        results.append({nm: np.asarray(out_arrs[i]).reshape(
            NCORE, *out_avals[i].shape)[c] for i, nm in enumerate(out_names)})
    return results


def _stage_inputs(in_maps):
    """Start async sharded uploads so the transfer overlaps _build()."""
    try:
        import jax
        from jax.sharding import Mesh, NamedSharding, PartitionSpec
        devices = jax.devices()[:NCORE]
        mesh = Mesh(np.array(devices), ("core",))
        sh = NamedSharding(mesh, PartitionSpec("core"))
        staged = {}
        for nm in in_maps[0]:
            cat = np.concatenate([np.asarray(in_maps[c][nm])
                                  for c in range(NCORE)], axis=0)
            staged[nm] = jax.device_put(cat, sh)
        _STATE["staged_inputs"] = staged
    except Exception:
        _STATE["staged_inputs"] = None


def kernel(**inputs):
    global LAST_HW_EXEC_NS

    _enable_jax_cache()
    th = None
    if "nc" not in _STATE:
        import threading
        holder = {}

        def _bg():
            holder["nc"] = _build()

        th = threading.Thread(target=_bg)
        th.start()
    in_maps = _prep_inputs(inputs)
    if th is not None:
        th.join()
        _STATE["nc"] = holder["nc"]
    nc = _STATE["nc"]

    t0 = time.time()
    try:
        results = _run_nodonate(nc, in_maps)
    except Exception:
        from concourse.bass_utils import run_bass_kernel_spmd
        results = run_bass_kernel_spmd(nc, in_maps, list(range(NCORE))).results
    LAST_HW_EXEC_NS = int((time.time() - t0) * 1e9)
    _STATE["res"] = type("R", (), {"results": results})()

    ys = np.concatenate([np.asarray(r["ys"], np.float32) for r in results])
    return ys.reshape(B, C, H, W).astype(np.float32)
